# revision 51
# baseline (speedup 1.0000x reference)
"""Trainium2 Bass kernel for nn_ClassificationModel.

Pipeline: frame-level bi-RNN (2 layers) -> utterance bi-GRU (2 layers) -> FC.
Data-parallel across 8 NeuronCores (2 utterances/core, no collectives).

Key idea vs the serial baseline: the GRU over F=512 frames is computed as
S=16 parallel chunks per (utterance, direction) with a W=16-step warmup
(GRU state error from a cold start decays geometrically, validated to
rel-err ~4.5e-3 overall), so each layer runs 48 serial steps instead of 512
with all chunks batched as matmul columns.

Layouts:
 - frames / GRU h buffers are "residue-major": col(p) for padded frame
   position p = (p % L)*(S+2)*U + (p//L + 1)*U + u.  Every GRU step then
   touches one contiguous Q=S*U block, and bulk input-preactivation matmuls
   write straight into PSUM wave tiles.
 - PSUM wave tile (4 banks): [r_f r_b | z_f z_b | n_f n_b | hn_f hn_b],
   each region 8 steps x Q.  Gate biases come from one small "indicator"
   matmul per bank; virtual (padded) columns get z=+30 so h stays exactly 0.
"""
import os
import sys
from contextlib import ExitStack

import numpy as np

sys.path.insert(0, '/opt/trn_rl_repo')

import concourse.bass as bass          # noqa: E402
import concourse.tile as tile          # noqa: E402
import concourse.mybir as mybir        # noqa: E402
from concourse import bacc             # noqa: E402
from concourse.bass_utils import run_bass_kernel_spmd  # noqa: E402

F32 = mybir.dt.float32
BF16 = mybir.dt.bfloat16
AF = mybir.ActivationFunctionType
ALU = mybir.AluOpType

B, F, T, M, H, C = 16, 512, 32, 128, 128, 61
NCORES, U = 8, 2
N = U * F                  # 1024 frame-cols per core
FCHUNK = 2
CC = N // FCHUNK           # 512 cols per x chunk

S = 64                     # GRU chunks per (utt, dir)
L = F // S                 # 8 frames per chunk
W = 8                      # warmup steps
NSTEP = L + W              # 16 steps per layer-direction
Q = S * U                  # 128 batch cols per dir-step
SLOTS = S + 2              # chunk slots incl left/right pad
GW = L * SLOTS * U         # 1056 cols of residue-major buffers
WAVE = 2
NWAVE = NSTEP // WAVE      # 8
REG = WAVE * Q             # 256 psum cols per (gate, dir) region
BANK = 2 * REG             # 512 = one psum bank

_cache = {}


def _col(p):
    """residue-major column of padded position p (u=0)."""
    return ((p % L) * SLOTS + (p // L) + 1) * U


def _rf(tau):
    return (L - W + tau) % L


def _rb(tau):
    return (L + W - 1 - tau) % L


def _blk_f(tau):
    """fw h/input block start col at step tau (length Q)."""
    slot0 = 0 if tau < W else 1
    return (_rf(tau) * SLOTS + slot0) * U


def _blk_b(tau):
    slot0 = 2 if tau < W else 1
    return (_rb(tau) * SLOTS + slot0) * U


def _build_program():
    nc = bacc.Bacc("TRN2", target_bir_lowering=False, debug=False)

    def din(name, shape, dt=F32):
        return nc.dram_tensor(name, shape, dt, kind="ExternalInput").ap()

    xT = din("xT", [FCHUNK, M, T, CC], BF16)
    NWB = 47
    # partition-major [128, NWB, 128] so each partition's data is one
    # contiguous 12KB DMA descriptor (row-major [NWB,128,128] needed 6016
    # 256B descriptors and took ~14us, descriptor-rate-bound).
    wb = din("wb", [128, NWB, 128], BF16)
    b0 = din("b0", [2, 128, 1])
    b1 = din("b1", [2, 128, 1])
    indic = din("indic", [128, 8, BANK], BF16)
    fcw = din("fcw", [128, 2, C], BF16)
    fcb = din("fcb", [C, 1])
    logits = nc.dram_tensor("logits", [C, GW], BF16, kind="ExternalOutput").ap()
    dbg = {}
    if os.environ.get("KDBG", "0") == "1":
        for nm in ["d_frames_f", "d_frames_b", "d_g0f", "d_g0b",
                   "d_g1f", "d_g1b"]:
            dbg[nm] = nc.dram_tensor(nm, [128, GW], BF16,
                                     kind="ExternalOutput").ap()

    with tile.TileContext(nc) as tc, ExitStack() as ctx:
        cpool = ctx.enter_context(tc.tile_pool(name="consts", bufs=1))

        def dram_ap(base_ap, offset, dims):
            return bass.AP(base_ap.tensor, offset,
                           [list(d) for d in dims], None,
                           base_ap.runtime_checks,
                           base_ap.dep_tracking_offset)

        # one bundled DMA for all [128,128] weight matrices.  Weights go on
        # the ACT engine's HW-DGE queue, indicators/fc on the gpsimd queue,
        # so the x DMA (sync queue) streams in parallel from t=0.
        # split the weight DMA: the frame phase only needs mats 0..8, so
        # that piece (0.29MB) lands ~5us before the full 1.5MB would —
        # the GRU mats stream afterwards, long before they're needed.
        NFR = 9
        wbt = cpool.tile([128, NWB * 128], BF16, tag="wb", name="wb")
        nc.scalar.dma_start(
            wbt[:, 0:NFR * 128],
            dram_ap(wb, 0, [[NWB * 128, 128], [1, NFR * 128]]))

        def wsl(k):
            return wbt[:, k * 128:(k + 1) * 128]

        w0ih_t = [wsl(0 + d) for d in range(2)]
        w0hh_t = [wsl(2 + d) for d in range(2)]
        w1ih_t = [[wsl(4 + 2 * d + k) for k in range(2)] for d in range(2)]
        w1hh0_t = wsl(8)
        gwih_t = [[[[wsl(9 + 12 * l + 6 * d + 2 * g + k)
                     for k in range(2)] for g in range(3)] for d in range(2)]
                  for l in range(2)]
        gwhh_t = [[[wsl(33 + 6 * l + 3 * d + g)
                    for g in range(3)] for d in range(2)] for l in range(2)]
        biasB_t = [wsl(45 + l) for l in range(2)]

        # small consts early on the scalar HW-DGE queue (the gpsimd queue is
        # software-DGE and pathologically slow), then the big indicator
        # table (only needed by the GRU phase, ~150us in).
        b0t = cpool.tile([128, 2], F32, tag="b0", name="b0")
        nc.scalar.dma_start(b0t[:].rearrange("p (k c) -> p k c", c=1),
                            dram_ap(b0, 0, [[1, 128], [128, 2], [1, 1]]))
        b0_t = [b0t[:, d:d + 1] for d in range(2)]
        b1t = cpool.tile([128, 2], F32, tag="b1", name="b1")
        nc.scalar.dma_start(b1t[:].rearrange("p (k c) -> p k c", c=1),
                            dram_ap(b1, 0, [[1, 128], [128, 2], [1, 1]]))
        b1_t = [b1t[:, d:d + 1] for d in range(2)]
        fcbt = cpool.tile([C, 1], F32, tag="fcb", name="fcb")
        nc.scalar.dma_start(fcbt[:], fcb)
        fcb_t = fcbt

        # GRU weights after the small frame-phase consts
        nc.scalar.dma_start(
            wbt[:, NFR * 128:],
            dram_ap(wb, NFR * 128,
                    [[NWB * 128, 128], [1, (NWB - NFR) * 128]]))

        fcwt = cpool.tile([128, 2 * C], BF16, tag="fcw", name="fcw")
        nc.scalar.dma_start(
            fcwt[:],
            dram_ap(fcw, 0, [[2 * C, 128], [1, 2 * C]]))
        fcw_t = [fcwt[:, k * C:(k + 1) * C] for k in range(2)]

        indt = cpool.tile([128, 8 * BANK], BF16, tag="ind", name="ind")
        nc.scalar.dma_start(
            indt[:],
            dram_ap(indic, 0, [[8 * BANK, 128], [1, 8 * BANK]]))
        indic_t = [indt[:, i * BANK:(i + 1) * BANK] for i in range(8)]

        # frames / gout hold both directions in ONE tile [128, 2*GW]
        # (dir-major) so GRU elementwise ops can fuse both directions into
        # single instructions with a strided dir axis.
        persist = ctx.enter_context(tc.tile_pool(name="persist", bufs=1))
        frames_t = persist.tile([128, 2 * GW], BF16, tag="frames",
                                name="frames")
        gout_t = [persist.tile([128, 2 * GW], BF16, tag=f"gout{l}",
                               name=f"gout{l}") for l in range(2)]
        frames = [frames_t[:, d * GW:(d + 1) * GW] for d in range(2)]
        gout = [[gout_t[l][:, d * GW:(d + 1) * GW] for d in range(2)]
                for l in range(2)]
        for tl in [frames_t] + gout_t:
            nc.vector.memset(tl[:], 0.0)

        def ap3(t_ap, dims):
            """free-dims override: dims = [[stride, n], ...] after partition."""
            return bass.AP(t_ap.tensor, t_ap.offset,
                           [list(t_ap.ap[0])] + [list(d) for d in dims],
                           None, t_ap.runtime_checks, t_ap.dep_tracking_offset)

        # ================= frame phase =================
        # p1: ch0 L0 (fw+bw chains); p2: ch1 L0 + ch0 L1f; p3: ch1 L1f
        # (column-split into 2 half-chains) + L1b single steps.
        with ExitStack() as fctx:
            xpool = fctx.enter_context(tc.tile_pool(name="xchunk", bufs=1))
            o0pool = fctx.enter_context(tc.tile_pool(name="o0", bufs=2))
            h1pool = fctx.enter_context(tc.tile_pool(name="h1", bufs=2))
            fpsp = fctx.enter_context(
                tc.tile_pool(name="fps", bufs=6, space="PSUM"))

            def fps_tile():
                return fpsp.tile([128, CC], F32, tag="ps", name="ps")

            def tsl(t):
                return slice(t * CC, (t + 1) * CC)

            def fr_out_ap(d, ch, half=None):
                # scatter [128, CC] -> frames[d] residue-major, chunk ch
                # col j = sb*32*U + r*U + u  (f = ch*256 + sb*32 + r)
                f0 = ch * (CC // U)
                nsb = CC // U // L
                if half is not None:
                    f0 += half * (CC // U // 2)
                    nsb //= 2
                base = frames[d][:, _col(f0):]
                return ap3(base, [[U, nsb], [SLOTS * U, L], [1, U]])

            def dma_chunk(ch):
                xt = xpool.tile([128, T * CC], BF16, tag="x", name=f"x{ch}")
                # 2-slice transfers ordered so the bw chain (t=T-1 down) and
                # fw chain (t=0 up) both have input ~1.5us after DMA start.
                order = []
                for k in range(T // 4):
                    order += [T - 2 - 2 * k, 2 * k]
                for t0 in order:
                    nc.sync.dma_start(
                        xt[:, t0 * CC:(t0 + 2) * CC],
                        dram_ap(xT, (ch * M * T + t0) * CC,
                                [[T * CC, 128], [1, 2 * CC]]))
                o0f = o0pool.tile([128, T * CC], BF16, tag="o0f",
                                  name=f"o0f{ch}")
                o0b = o0pool.tile([128, T * CC], BF16, tag="o0b",
                                  name=f"o0b{ch}")
                return xt, o0f, o0b

            def l0_step(ch, xt, o0f, o0b, t):
                tb = T - 1 - t
                ps = fps_tile()
                nc.tensor.matmul(ps[:], w0ih_t[0][:], xt[:, tsl(t)],
                                 start=True, stop=(t == 0))
                if t > 0:
                    nc.tensor.matmul(ps[:], w0hh_t[0][:], o0f[:, tsl(t - 1)],
                                     start=False, stop=True)
                nc.scalar.activation(o0f[:, tsl(t)], ps[:], AF.Tanh,
                                     bias=b0_t[0][:])
                ps = fps_tile()
                nc.tensor.matmul(ps[:], w0ih_t[1][:], xt[:, tsl(tb)],
                                 start=True, stop=(t == 0))
                if t > 0:
                    nc.tensor.matmul(ps[:], w0hh_t[1][:], o0b[:, tsl(tb + 1)],
                                     start=False, stop=True)
                nc.scalar.activation(o0b[:, tsl(tb)], ps[:], AF.Tanh,
                                     bias=b0_t[1][:])

            def l1f_step(ch, o0f, o0b, t, hprev, half=None, pool=0):
                if half is None:
                    c0, cn = 0, CC
                else:
                    c0, cn = half * (CC // 2), CC // 2
                pst = fps_tile()
                ps = pst[:, 0:cn]
                nc.tensor.matmul(ps, w1ih_t[0][0][:],
                                 o0f[:, t * CC + c0:t * CC + c0 + cn],
                                 start=True, stop=False)
                nc.tensor.matmul(ps, w1ih_t[0][1][:],
                                 o0b[:, t * CC + c0:t * CC + c0 + cn],
                                 start=False, stop=(t == 0))
                if t > 0:
                    nc.tensor.matmul(ps, w1hh0_t[:], hprev[:],
                                     start=False, stop=True)
                if t == T - 1:
                    psv = ps.rearrange("p (a b c) -> p a b c",
                                       a=cn // U // L, b=L)
                    nc.scalar.activation(fr_out_ap(0, ch, half), psv,
                                         AF.Tanh, bias=b1_t[0][:])
                    return None
                h1 = h1pool.tile([128, cn], BF16, tag=f"h1_{pool}",
                                 name="h1")
                nc.scalar.activation(h1[:], ps, AF.Tanh, bias=b1_t[0][:])
                return h1

            def l1b_step(ch, o0f, o0b, pool):
                ps = fps_tile()
                nc.tensor.matmul(ps[:], w1ih_t[1][0][:], o0f[:, tsl(T - 1)],
                                 start=True, stop=False)
                nc.tensor.matmul(ps[:], w1ih_t[1][1][:], o0b[:, tsl(T - 1)],
                                 start=False, stop=True)
                psv = ps[:].rearrange("p (a b c) -> p a b c",
                                      a=CC // U // L, b=L)
                nc.scalar.activation(fr_out_ap(1, ch), psv, AF.Tanh,
                                     bias=b1_t[1][:])

            # ---- p1: ch0 L0 ----
            xt0, o0f0, o0b0 = dma_chunk(0)
            for t in range(T):
                l0_step(0, xt0, o0f0, o0b0, t)
            # ---- p2: ch1 L0 + ch0 L1f ----
            xt1, o0f1, o0b1 = dma_chunk(1)
            h0 = None
            for t in range(T):
                l0_step(1, xt1, o0f1, o0b1, t)
                h0 = l1f_step(0, o0f0, o0b0, t, h0, pool=0)
            l1b_step(0, o0f0, o0b0, pool=0)
            # ---- p3: ch1 L1f (2 half-chains) + ch1 L1b ----
            h1a, h1b = None, None
            for t in range(T):
                h1a = l1f_step(1, o0f1, o0b1, t, h1a, half=0, pool=1)
                h1b = l1f_step(1, o0f1, o0b1, t, h1b, half=1, pool=2)
            l1b_step(1, o0f1, o0b1, pool=3)

        # ================= GRU layers =================
        # psum wave layout (4 banks, per-direction to keep dep tracking,
        # which is bank-granular, from cross-linking the fw/bw chains):
        #   bank 2d+0: [r_d (REG) | z_d (REG)]
        #   bank 2d+1: [n_d (REG) | hn_d (REG)]
        def off_r(d):
            return 2 * d * BANK

        def off_z(d):
            return 2 * d * BANK + REG

        def off_n(d):
            return (2 * d + 1) * BANK

        def off_hn(d):
            return (2 * d + 1) * BANK + REG

        with ExitStack() as gctx:
            gps = gctx.enter_context(
                tc.tile_pool(name="gps", bufs=2, space="PSUM"))
            sp = gctx.enter_context(tc.tile_pool(name="gsp", bufs=8))

            def make_prep(l):
                xin = frames if l == 0 else gout[0]
                waves = [None] * NWAVE

                def prep_piece(w, pc, l=l, xin=xin, waves=waves):
                    # pc 0: alloc tile + bias matmuls; pc 1..6: gi matmuls
                    # for (d, g) = divmod(pc-1, 3)
                    if pc == 0:
                        pw = gps.tile([128, 4 * BANK], F32, tag="wv",
                                      name="wv")
                        waves[w] = pw
                        typ = 0 if (w * WAVE) < W else 1
                        for bk in range(4):
                            nc.tensor.matmul(
                                pw[:, bk * BANK:(bk + 1) * BANK],
                                biasB_t[l][:], indic_t[4 * typ + bk][:],
                                start=True, stop=False)
                        return
                    pw = waves[w]
                    d, g = divmod(pc - 1, 3)
                    tau0 = w * WAVE
                    c0 = _blk_f(tau0) if d == 0 else _blk_b(tau0 + WAVE - 1)
                    mv = [ap3(xin[k][:, c0:], [[SLOTS * U, WAVE], [1, Q]])
                          for k in range(2)]
                    off = (off_r(d), off_z(d), off_n(d))[g]
                    out = pw[:, off:off + REG].rearrange(
                        "p (s q) -> p s q", q=Q)
                    nc.tensor.matmul(out, gwih_t[l][d][g][0][:], mv[0],
                                     start=False, stop=False)
                    nc.tensor.matmul(out, gwih_t[l][d][g][1][:], mv[1],
                                     start=False, stop=(g == 2))
                return waves, prep_piece

            layer_prep = [make_prep(l) for l in range(2)]

            for l in range(2):
                go = gout[l]
                waves, prep_piece = layer_prep[l]

                for w in (0, 1):
                    for pc in range(7):
                        prep_piece(w, pc)

                # Both directions fused into single instructions per op via
                # a strided dir axis (psum: +2*BANK between dirs; gout: the
                # dir-major fused tile).  Halves the instruction count on
                # the serial chain (the scheduler's single-wait coalescing
                # made the dirs rendezvous anyway).
                for tau in range(NSTEP):
                    w, tm = tau // WAVE, tau % WAVE
                    pw = waves[w]
                    sl = [tm, WAVE - 1 - tm]          # psum step slot per dir
                    blk = [_blk_f(tau), _blk_b(tau)]
                    blkp = [_blk_f(tau - 1), _blk_b(tau - 1)]
                    pds = 2 * BANK + (sl[1] - sl[0]) * Q  # psum dir stride

                    def pfuse(off_fn):
                        o = off_fn(0) + sl[0] * Q
                        return ap3(pw[:, o:], [[pds, 2], [1, Q]])

                    if tau > 0:
                        for d in range(2):
                            hb = go[d][:, blkp[d]:blkp[d] + Q]
                            for g, off_fn in ((0, off_r), (1, off_z),
                                              (2, off_hn)):
                                o = off_fn(d) + sl[d] * Q
                                nc.tensor.matmul(pw[:, o:o + Q],
                                                 gwhh_t[l][d][g][:], hb,
                                                 start=False, stop=True)
                    # emit prep pieces of wave w+1 during wave w's steps so
                    # the aliased psum tile (wave w-1) is already drained.
                    if w >= 1 and w + 1 < NWAVE and tm >= 1:
                        for pc in range(7 * (tm - 1) // (WAVE - 1),
                                        7 * tm // (WAVE - 1)):
                            prep_piece(w + 1, pc)

                    rz = sp.tile([128, 4 * Q], F32, tag="rz", name="rz")
                    rzv = rz[:].rearrange("p (d a q) -> p d a q", a=2, q=Q)
                    inap = ap3(pw[:, off_r(0) + sl[0] * Q:],
                               [[pds, 2], [REG, 2], [1, Q]])
                    nc.scalar.activation(rzv, inap, AF.Sigmoid)
                    rz_r = ap3(rz[:], [[2 * Q, 2], [1, Q]])
                    rz_z = ap3(rz[:, Q:], [[2 * Q, 2], [1, Q]])

                    zc = sp.tile([128, 2 * Q], F32, tag="zc", name="zc")
                    zcv = zc[:].rearrange("p (d q) -> p d q", q=Q)
                    nc.gpsimd.tensor_scalar(zcv, rz_z, -1.0, 1.0,
                                            ALU.mult, ALU.add)
                    t1 = sp.tile([128, 2 * Q], F32, tag="t1", name="t1")
                    t1v = t1[:].rearrange("p (d q) -> p d q", q=Q)
                    nc.vector.tensor_mul(t1v, rz_r, pfuse(off_hn))
                    t2 = sp.tile([128, 2 * Q], F32, tag="t2", name="t2")
                    t2v = t2[:].rearrange("p (d q) -> p d q", q=Q)
                    nc.vector.tensor_add(t2v, t1v, pfuse(off_n))
                    n_ = sp.tile([128, 2 * Q], F32, tag="n_", name="n_")
                    nc.scalar.activation(n_[:], t2[:], AF.Tanh)
                    n_v = n_[:].rearrange("p (d q) -> p d q", q=Q)

                    go_prev = ap3(gout_t[l][:, blkp[0]:],
                                  [[GW + blkp[1] - blkp[0], 2], [1, Q]])
                    go_new = ap3(gout_t[l][:, blk[0]:],
                                 [[GW + blk[1] - blk[0], 2], [1, Q]])
                    if tau > 0:
                        p_ = sp.tile([128, 2 * Q], F32, tag="p_", name="p_")
                        pv = p_[:].rearrange("p (d q) -> p d q", q=Q)
                        nc.gpsimd.tensor_mul(pv, rz_z, go_prev)
                        q_ = sp.tile([128, 2 * Q], F32, tag="q_", name="q_")
                        qv = q_[:].rearrange("p (d q) -> p d q", q=Q)
                        nc.vector.tensor_mul(qv, zcv, n_v)
                        nc.vector.tensor_add(go_new, pv, qv)
                    else:
                        nc.vector.tensor_mul(go_new, zcv, n_v)
                    # HAM keep-warm: junk accumulations into the RETIRED
                    # wave's psum banks (WAR-ordered after all readers,
                    # overwritten by the next start=True bias before any
                    # read — values never observed).  8 x N=256 on the
                    # light tm==0 steps to approach sustained PE activity.
                    if tm == 0 and w >= 1:
                        for dk in range(16):
                            o = ((dk % 8) // 2) * BANK + (dk % 2) * REG
                            nc.tensor.matmul(
                                waves[w - 1][:, o:o + REG],
                                w0ih_t[0][:], frames_t[:, 0:REG],
                                start=False, stop=False)

        # ================= FC =================
        with ExitStack() as fc_ctx:
            fcp = fc_ctx.enter_context(
                tc.tile_pool(name="fcp", bufs=1, space="PSUM"))
            lpool = fc_ctx.enter_context(tc.tile_pool(name="lsb", bufs=1))
            lsb = lpool.tile([C, GW], BF16, tag="lsb", name="lsb")
            ps = fcp.tile([C, GW], F32, tag="fcps", name="fcps")
            splits = [(0, 512), (512, 512), (1024, GW - 1024)]
            dma_engs = [nc.sync, nc.scalar, nc.gpsimd]
            for i, (c0, cn) in enumerate(splits):
                for k in range(2):
                    nc.tensor.matmul(ps[:, c0:c0 + cn], fcw_t[k][:],
                                     gout[1][k][:, c0:c0 + cn],
                                     start=(k == 0), stop=(k == 1))
                nc.scalar.activation(lsb[:, c0:c0 + cn], ps[:, c0:c0 + cn],
                                     AF.Identity, bias=fcb_t[:])
                # stream each finished piece on its own DMA queue
                dma_engs[i].dma_start(
                    dram_ap(logits, c0, [[GW, C], [1, cn]]),
                    lsb[:, c0:c0 + cn])
            if dbg:
                nc.sync.dma_start(dbg["d_frames_f"], frames[0][:])
                nc.sync.dma_start(dbg["d_frames_b"], frames[1][:])
                nc.sync.dma_start(dbg["d_g0f"], gout[0][0][:])
                nc.sync.dma_start(dbg["d_g0b"], gout[0][1][:])
                nc.sync.dma_start(dbg["d_g1f"], gout[1][0][:])
                nc.sync.dma_start(dbg["d_g1b"], gout[1][1][:])

    nc.compile()
    return nc


def _prep_common(inp):
    import ml_dtypes
    bf = ml_dtypes.bfloat16
    f32 = np.float32
    c = {}
    wb = np.zeros((47, 128, 128), f32)
    for d in range(2):
        wb[0 + d] = inp["rnn1_l0_Wih"][d].T
        wb[2 + d] = inp["rnn1_l0_Whh"][d].T
    c["b0"] = np.ascontiguousarray(
        (inp["rnn1_l0_bih"] + inp["rnn1_l0_bhh"])[:, :, None], dtype=f32)
    w1 = np.stack([inp["rnn1_l1_Wih"][d].T for d in range(2)])
    w1 = w1.reshape(2, 2, 128, 128)
    for d in range(2):
        for k in range(2):
            wb[4 + 2 * d + k] = w1[d, k]
    wb[8] = inp["rnn1_l1_Whh"][0].T
    c["b1"] = np.ascontiguousarray(
        (inp["rnn1_l1_bih"] + inp["rnn1_l1_bhh"])[:, :, None], dtype=f32)

    for l in range(2):
        wih = np.asarray(inp[f"gru_l{l}_Wih"], f32)
        whh = np.asarray(inp[f"gru_l{l}_Whh"], f32)
        bih = np.asarray(inp[f"gru_l{l}_bih"], f32)
        bhh = np.asarray(inp[f"gru_l{l}_bhh"], f32)
        for d in range(2):
            for g in range(3):
                wt = wih[d, g * H:(g + 1) * H, :].T.reshape(2, 128, 128)
                for k in range(2):
                    wb[9 + 12 * l + 6 * d + 2 * g + k] = wt[k]
                wb[33 + 6 * l + 3 * d + g] = whh[d, g * H:(g + 1) * H, :].T
        Bm = np.zeros((128, 128), f32)
        for d in range(2):
            Bm[0 + d] = bih[d, 0:H] + bhh[d, 0:H]          # r
            Bm[2 + d] = bih[d, H:2 * H] + bhh[d, H:2 * H]  # z
            Bm[4 + d] = bih[d, 2 * H:]                     # n
            Bm[6 + d] = bhh[d, 2 * H:]                     # hn
        Bm[8] = 30.0
        wb[45 + l] = Bm
    # partition-major for single-descriptor-per-partition DMA
    c["wb"] = np.ascontiguousarray(wb.transpose(1, 0, 2)).astype(bf)

    # indicator patterns [type(2) x bank(4), 128, BANK]
    # bank 2d+0 = [r_d | z_d]; bank 2d+1 = [n_d | hn_d]
    ind = np.zeros((8, 128, BANK), f32)
    for typ in range(2):
        for bk in range(4):
            pat = ind[4 * typ + bk]
            d, kind = bk >> 1, bk & 1
            for j in range(BANK):
                half = j // REG
                rel = j % Q
                virt = (typ == 0) and (
                    (d == 0 and rel < U) or (d == 1 and rel >= Q - U))
                if virt:
                    row = 8 if (kind == 0 and half == 1) else 9
                else:
                    row = (0, 2, 4, 6)[2 * kind + half] + d
                if row < 9:
                    pat[row, j] = 1.0
    import ml_dtypes as md
    c["indic"] = np.ascontiguousarray(
        ind.transpose(1, 0, 2)).astype(md.bfloat16)
    c["fcw"] = np.ascontiguousarray(
        np.asarray(inp["fc_W"], f32).T.reshape(2, 128, C).transpose(
            1, 0, 2)).astype(md.bfloat16)
    c["fcb"] = np.ascontiguousarray(np.asarray(inp["fc_b"], f32)[:, None])
    return c


def _shard_x(x):
    import ml_dtypes
    xs = np.asarray(x, dtype=np.float32).reshape(B, F, T, M)
    xs = xs.astype(ml_dtypes.bfloat16)
    shards = []
    for cidx in range(NCORES):
        xc = xs[U * cidx:U * cidx + U]               # [U, F, T, M]
        xt = xc.transpose(2, 3, 1, 0)                # [T, M, F, U]
        xt = xt.reshape(T, M, FCHUNK, F // FCHUNK, U)
        xt = xt.transpose(2, 1, 0, 3, 4).reshape(FCHUNK, M, T, CC)
        shards.append(np.ascontiguousarray(xt))
    return shards


def _install_ntff_hook_shim():
    """Provide antenv.axon_hooks (missing in this image) so trace=True can
    capture NTFF profiles through the axon PJRT .so."""
    import types
    import ctypes
    import contextlib
    if "antenv.axon_hooks" in sys.modules:
        return
    so_path = "/opt/axon/libaxon_pjrt.so"
    if not os.path.exists(so_path):
        return
    lib = ctypes.CDLL(so_path)
    if not hasattr(lib, "axon_start_nrt_profile"):
        return
    lib.axon_start_nrt_profile.argtypes = [
        ctypes.POINTER(ctypes.c_int64), ctypes.c_size_t]
    lib.axon_start_nrt_profile.restype = ctypes.c_int64
    lib.axon_stop_nrt_profile.argtypes = [ctypes.c_char_p]
    lib.axon_stop_nrt_profile.restype = ctypes.c_int64

    @contextlib.contextmanager
    def _hook(output_dir, device_ids):
        import jax
        jax.devices()
        if device_ids:
            ids = (ctypes.c_int64 * len(device_ids))(*device_ids)
            rc = lib.axon_start_nrt_profile(ids, len(device_ids))
        else:
            rc = lib.axon_start_nrt_profile(None, 0)
        if rc != 0:
            raise RuntimeError(f"axon_start_nrt_profile rc={rc}")
        try:
            yield
        finally:
            n = lib.axon_stop_nrt_profile(str(output_dir).encode())
            print(f"ntff profile: {n} file(s) -> {output_dir}")

    mod = types.ModuleType("antenv.axon_hooks")
    mod.get_axon_ntff_profile_hook = lambda: _hook
    mod.set_axon_ntff_profile_hook = lambda h: None
    sys.modules["antenv.axon_hooks"] = mod


def kernel(**inputs):
    inputs = {k: np.asarray(v) for k, v in inputs.items()}
    if "nc" not in _cache:
        _cache["nc"] = _build_program()
    nc = _cache["nc"]

    common = _prep_common(inputs)
    shards = _shard_x(inputs["x"])
    in_maps = []
    for cidx in range(NCORES):
        m = {"xT": shards[cidx]}
        m.update(common)
        in_maps.append(m)

    trace = os.environ.get("KERNEL_TRACE", "0") == "1"
    if trace:
        _install_ntff_hook_shim()
    res = run_bass_kernel_spmd(nc, in_maps, list(range(NCORES)), trace=trace)
    _cache["last_results"] = res

    logits_all = np.empty((B, F, C), np.float32)
    for cidx in range(NCORES):
        lg = np.asarray(res.results[cidx]["logits"]).astype(np.float32)
        lg = lg.reshape(C, L, SLOTS, U)[:, :, 1:S + 1, :]  # [C, L, S, U]
        # f = s*L + r
        lg = lg.transpose(3, 2, 1, 0)                  # [U, S, L, C]
        logits_all[U * cidx:U * cidx + U] = lg.reshape(U, F, C)
    Ls = np.asarray(inputs["lengths"]).astype(np.int64)
    return np.concatenate([logits_all[i, :Ls[i]] for i in range(B)], axis=0)

